# revision 2
# baseline (speedup 1.0000x reference)
"""Trainium2 Bass kernel: Sudoku information gain H(before) - H(after).

Self-contained: builds one SPMD Bass/Tile program, shards the batch
across 8 NeuronCores (pure data parallel), runs via
run_bass_kernel_spmd, and reassembles the full [B] output.

Algorithm per 9x9 grid (values 0..9, 0 = empty):
  encode each cell x as e = 1024 >> x on ScalarE (Exp activation):
    bit 10 <=> empty, bit (10-v) <=> value v.
  Bitwise-OR reductions of e give row/col/box presence masks on the
  DVE (u16, mostly 2x/4x-mode ops).  Per cell m = row|col|box|e_own.
  SWAR popcount: s = m - ((m>>1)&0x155) pair counts, c = base-16 digit
  compaction, pc = (c*0x1110)>>12 digit sum via u16 wrap multiply.
  fw = 8*empty comes from a parallel Relu(-16x+8) activation on
  ScalarE; u = max(fw - pc, 0) so Ln(u+1) = ln(max(9-forbidden,1)) for
  empty cells and 0 for filled ones.  Ln emits fp16; the before/after
  difference is taken per cell (2x-mode TT) so each tile needs only a
  single f32 TensorReduce; a 1/ln2 scale produces the final bits.
"""

import math
from contextlib import ExitStack

import numpy as np

import concourse.bass as bass
import concourse.bacc as bacc
import concourse.tile as tile
from concourse import mybir
from concourse.alu_op_type import AluOpType
from concourse.bass_utils import run_bass_kernel_spmd

F32 = mybir.dt.float32
F16 = mybir.dt.float16
U16 = mybir.dt.uint16
I16 = mybir.dt.int16

LN2 = math.log(2.0)
LOG1024 = math.log(1024.0)
EPS = 1e-5

OR = AluOpType.bitwise_or
AND = AluOpType.bitwise_and
ADD = AluOpType.add
SUB = AluOpType.subtract
MULT = AluOpType.mult
MAX = AluOpType.max
SHR = AluOpType.logical_shift_right

N_CORES = 8
BATCH = 262144
PER_CORE = BATCH // N_CORES  # 32768
F = 16  # grids per partition per tile

ACT_FW = True    # fw = 8*empty via Relu(-16x+8) on the Act engine
FP16_LN = True   # Ln output fp16, per-cell diff, single reduce per tile
MULT_PC = False  # u16 mult SATURATES on HW: use mod-15 digit sum
WP_BUFS = 3
IOP_BUFS = 3


def _masks_and_m(nc, wp, e, F):
    ve = e[:]
    e4 = ve.rearrange("p (f r c) -> p f r c", f=F, r=9, c=9)
    e5 = ve.rearrange("p (f b i c) -> p f b i c", f=F, b=3, i=3, c=9)

    t3 = wp.tile([128, F * 27], U16, tag="t3")
    t3v = t3[:].rearrange("p (f r b) -> p f r b", f=F, r=9, b=3)
    nc.vector.tensor_tensor(t3v, e4[:, :, :, 0:3], e4[:, :, :, 3:6], op=OR)
    nc.vector.tensor_tensor(t3v, t3v, e4[:, :, :, 6:9], op=OR)

    row = wp.tile([128, F * 9], U16, tag="row")
    rv = row[:].rearrange("p (f r) -> p f r", f=F, r=9)
    t3b = t3[:].rearrange("p (f r b) -> p f r b", f=F, r=9, b=3)
    nc.vector.tensor_tensor(rv, t3b[:, :, :, 0], t3b[:, :, :, 1], op=OR)
    nc.vector.tensor_tensor(rv, rv, t3b[:, :, :, 2], op=OR)

    bcol = wp.tile([128, F * 27], U16, tag="bcol")
    bv = bcol[:].rearrange("p (f b c) -> p f b c", f=F, b=3, c=9)
    nc.vector.tensor_tensor(bv, e5[:, :, :, 0, :], e5[:, :, :, 1, :], op=OR)
    nc.vector.tensor_tensor(bv, bv, e5[:, :, :, 2, :], op=OR)

    col = wp.tile([128, F * 9], U16, tag="col")
    cv = col[:].rearrange("p (f c) -> p f c", f=F, c=9)
    bc3 = bcol[:].rearrange("p (f b c) -> p f b c", f=F, b=3, c=9)
    nc.vector.tensor_tensor(cv, bc3[:, :, 0, :], bc3[:, :, 1, :], op=OR)
    nc.vector.tensor_tensor(cv, cv, bc3[:, :, 2, :], op=OR)

    box = wp.tile([128, F * 9], U16, tag="box")
    xv = box[:].rearrange("p (f b k) -> p f b k", f=F, b=3, k=3)
    bc4 = bcol[:].rearrange("p (f b k i) -> p f b k i", f=F, b=3, k=3, i=3)
    nc.vector.tensor_tensor(xv, bc4[:, :, :, :, 0], bc4[:, :, :, :, 1], op=OR)
    nc.vector.tensor_tensor(xv, xv, bc4[:, :, :, :, 2], op=OR)

    # q[f, r, bc] = row[f, r] | box[f, br(r), bc]
    q = wp.tile([128, F * 27], U16, tag="q")
    qv = q[:].rearrange("p (f b i k) -> p f b i k", f=F, b=3, i=3, k=3)
    rv3 = row[:].rearrange("p (f b i) -> p f b i", f=F, b=3, i=3)
    xv3 = box[:].rearrange("p (f b k) -> p f b k", f=F, b=3, k=3)
    for ir in range(3):
        nc.vector.tensor_tensor(
            qv[:, :, :, ir, :],
            rv3[:, :, :, ir].unsqueeze(3).broadcast_to((128, F, 3, 3)),
            xv3,
            op=OR,
        )

    # m[f, r, c] = q[f, r, bc(c)] | col[f, c]
    m = wp.tile([128, F * 81], U16, tag="m")
    mv = m[:].rearrange("p (f r b i) -> p f r b i", f=F, r=9, b=3, i=3)
    qv2 = q[:].rearrange("p (f r b) -> p f r b", f=F, r=9, b=3)
    cv2 = col[:].rearrange("p (f b i) -> p f b i", f=F, b=3, i=3)
    for bc in range(3):
        nc.vector.tensor_tensor(
            mv[:, :, :, bc, :],
            qv2[:, :, :, bc].unsqueeze(3).broadcast_to((128, F, 9, 3)),
            cv2[:, :, bc, :].unsqueeze(2).broadcast_to((128, F, 9, 3)),
            op=OR,
        )
    return m


def _entropy_from_m(nc, wp, m, e, fw_act, F):
    n = F * 81
    # pair counts: s = m - ((m>>1)&0x155); the bit-10 any-empty junk
    # never enters the masked fields (bit 0 of e is never set for 0..9)
    tmp = wp.tile([128, n], U16, tag="tmp")
    nc.vector.tensor_scalar(tmp[:], m[:], 1, 0x155, op0=SHR, op1=AND)
    s = wp.tile([128, n], U16, tag="s")
    nc.vector.tensor_tensor(s[:], m[:], tmp[:], op=SUB)

    # base-16 digit compaction: c = (s & 0x333) + ((s>>2) & 0x33)
    a = wp.tile([128, n], U16, tag="a")
    nc.vector.tensor_scalar(a[:], s[:], 2, 0x33, op0=SHR, op1=AND)
    c0 = wp.tile([128, n], U16, tag="c0")
    nc.vector.tensor_scalar(c0[:], s[:], 0x333, None, op0=AND)
    c = wp.tile([128, n], U16, tag="c")
    nc.vector.tensor_tensor(c[:], c0[:], a[:], op=ADD)

    if fw_act is not None:
        fw = fw_act
    else:
        fw = wp.tile([128, n], U16, tag="fw")
        nc.vector.tensor_scalar(fw[:], e[:], 7, 8, op0=SHR, op1=AND)

    if MULT_PC:
        # digit sum lands in bits 12-15 of c*0x1110 (u16 wrap multiply);
        # mult (arith) and shr (bitwise) cannot fuse in one tensor_scalar
        prod = wp.tile([128, n], U16, tag="prod")
        nc.vector.tensor_scalar(prod[:], c[:], 0x1110, None, op0=MULT)
        pc = wp.tile([128, n], U16, tag="pc")
        nc.vector.tensor_scalar(pc[:], prod[:], 12, None, op0=SHR)
    else:
        q15 = wp.tile([128, n], I16, tag="q15")
        nc.vector.tensor_scalar(q15[:], c[:], 1.0 / 15.0, -0.4999, op0=MULT, op1=ADD)
        v15 = wp.tile([128, n], I16, tag="v15")
        nc.vector.tensor_scalar(v15[:], q15[:], 15, None, op0=MULT)
        pc = wp.tile([128, n], I16, tag="pc")
        nc.vector.tensor_tensor(pc[:], c[:], v15[:], op=SUB)

    t = wp.tile([128, n], I16, tag="t")
    nc.vector.tensor_tensor(t[:], fw[:], pc[:], op=SUB)
    u = wp.tile([128, n], U16, tag="u")
    nc.vector.tensor_scalar(u[:], t[:], 0, None, op0=MAX)
    return u


def _emit(tc, out_ap, gb_ap, ga_ap, n_grids, F, reps=1):
    nc = tc.nc
    per_tile = 128 * F
    n_tiles = n_grids // per_tile
    ln_dt = F16 if FP16_LN else F32

    with ExitStack() as ctx:
        cp = ctx.enter_context(tc.tile_pool(name="const", bufs=1))
        iop = ctx.enter_context(tc.tile_pool(name="io", bufs=IOP_BUFS))
        wp = ctx.enter_context(tc.tile_pool(name="work", bufs=WP_BUFS))
        accp = ctx.enter_context(tc.tile_pool(name="acc", bufs=3))

        enc_bias = cp.tile([128, 1], F32, tag="enc_bias")
        nc.vector.memset(enc_bias[:], LOG1024 + EPS)
        fw_bias = cp.tile([128, 1], F32, tag="fw_bias")
        nc.vector.memset(fw_bias[:], 8.0)

        for i in [t for _ in range(reps) for t in range(n_tiles)]:
            encoded = {}
            fws = {}
            for key, src in (("b", gb_ap), ("a", ga_ap)):
                x = iop.tile([128, F * 81], F32, tag="x")
                view = src[i * per_tile : (i + 1) * per_tile, :].rearrange(
                    "(p f) c -> p (f c)", p=128
                )
                nc.sync.dma_start(x[:], view)

                e = wp.tile([128, F * 81], U16, tag="e" + key)
                nc.scalar.activation(
                    e[:],
                    x[:],
                    mybir.ActivationFunctionType.Exp,
                    bias=enc_bias[:],
                    scale=-LN2,
                )
                encoded[key] = e
                if ACT_FW:
                    fw = wp.tile([128, F * 81], U16, tag="fw" + key)
                    nc.scalar.activation(
                        fw[:],
                        x[:],
                        mybir.ActivationFunctionType.Relu,
                        bias=fw_bias[:],
                        scale=-16.0,
                    )
                    fws[key] = fw

            us = {}
            for key in ("b", "a"):
                e = encoded[key]
                m = _masks_and_m(nc, wp, e, F)
                us[key] = _entropy_from_m(nc, wp, m, e, fws.get(key), F)

            lns = {}
            for key in ("b", "a"):
                lnv = wp.tile([128, F * 81], ln_dt, tag="ln" + key)
                nc.scalar.activation(
                    lnv[:], us[key][:], mybir.ActivationFunctionType.Ln, bias=1.0
                )
                lns[key] = lnv

            if FP16_LN:
                lnd = wp.tile([128, F * 81], F16, tag="lnd")
                nc.vector.tensor_tensor(lnd[:], lns["b"][:], lns["a"][:], op=SUB)
                tot = accp.tile([128, F], F32, tag="tot")
                nc.vector.tensor_reduce(
                    tot[:],
                    lnd[:].rearrange("p (f c) -> p f c", f=F, c=81),
                    axis=mybir.AxisListType.X,
                    op=ADD,
                )
                nc.vector.tensor_scalar(tot[:], tot[:], 1.0 / LN2, None, op0=MULT)
                diff = tot
            else:
                tots = {}
                for key in ("b", "a"):
                    tot = accp.tile([128, F], F32, tag="tot" + key)
                    nc.vector.tensor_reduce(
                        tot[:],
                        lns[key][:].rearrange("p (f c) -> p f c", f=F, c=81),
                        axis=mybir.AxisListType.X,
                        op=ADD,
                    )
                    tots[key] = tot
                diff = accp.tile([128, F], F32, tag="diff")
                nc.vector.tensor_tensor(diff[:], tots["b"][:], tots["a"][:], op=SUB)
                nc.vector.tensor_scalar(diff[:], diff[:], 1.0 / LN2, None, op0=MULT)

            out_view = out_ap[i * per_tile : (i + 1) * per_tile].rearrange(
                "(p f) -> p f", p=128
            )
            nc.sync.dma_start(out_view, diff[:])


_PROGRAM_CACHE = {}


def _build_program(reps=1):
    key = (PER_CORE, F, reps, ACT_FW, FP16_LN, MULT_PC, WP_BUFS, IOP_BUFS)
    if key in _PROGRAM_CACHE:
        return _PROGRAM_CACHE[key]
    nc = bacc.Bacc("TRN2", target_bir_lowering=False, debug=False)
    gb = nc.dram_tensor("grid_before", [PER_CORE, 81], F32, kind="ExternalInput")
    ga = nc.dram_tensor("grid_after", [PER_CORE, 81], F32, kind="ExternalInput")
    out = nc.dram_tensor("out", [PER_CORE], F32, kind="ExternalOutput")
    with tile.TileContext(nc) as tc:
        _emit(tc, out.ap(), gb.ap(), ga.ap(), PER_CORE, F, reps=reps)
    nc.finalize()
    _PROGRAM_CACHE[key] = nc
    return nc


def run(grid_before, grid_after, trace=False, **trace_kwargs):
    gb = np.ascontiguousarray(
        np.asarray(grid_before, dtype=np.float32).reshape(BATCH, 81)
    )
    ga = np.ascontiguousarray(
        np.asarray(grid_after, dtype=np.float32).reshape(BATCH, 81)
    )
    nc = _build_program()
    in_maps = [
        {
            "grid_before": gb[k * PER_CORE : (k + 1) * PER_CORE],
            "grid_after": ga[k * PER_CORE : (k + 1) * PER_CORE],
        }
        for k in range(N_CORES)
    ]
    res = run_bass_kernel_spmd(
        nc, in_maps, list(range(N_CORES)), trace=trace, **trace_kwargs
    )
    out = np.concatenate([res.results[k]["out"] for k in range(N_CORES)])
    return out, res


def kernel(grid_before, grid_after):
    out, _ = run(grid_before, grid_after)
    return out


def bench(grid_before, grid_after, iters=250, warmup=3, reps=8):
    """Per-computation wall time with device-resident inputs.

    The NEFF repeats the full computation `reps` times back-to-back so
    the per-dispatch axon RPC cost (~1 ms) is amortized below the
    kernel's own execution time; `iters` pipelined dispatches amortize
    the fixed pipeline-fill cost.  Reported time = wall / (iters*reps).
    """
    import time

    import jax
    import concourse.mybir as mybir_
    from jax.sharding import Mesh, NamedSharding, PartitionSpec
    from jax.experimental.shard_map import shard_map
    from concourse.bass2jax import (
        _bass_exec_p,
        install_neuronx_cc_hook,
        partition_id_tensor,
    )

    install_neuronx_cc_hook()
    gb = np.ascontiguousarray(
        np.asarray(grid_before, dtype=np.float32).reshape(BATCH, 81)
    )
    ga = np.ascontiguousarray(
        np.asarray(grid_after, dtype=np.float32).reshape(BATCH, 81)
    )
    nc = _build_program(reps=reps)

    part_name = nc.partition_id_tensor.name if nc.partition_id_tensor else None
    in_names, out_names, out_avals, zero_outs = [], [], [], []
    for alloc in nc.m.functions[0].allocations:
        if not isinstance(alloc, mybir.MemoryLocationSet):
            continue
        name = alloc.memorylocations[0].name
        if alloc.kind == "ExternalInput":
            if name != part_name:
                in_names.append(name)
        elif alloc.kind == "ExternalOutput":
            out_names.append(name)
            shape = tuple(alloc.tensor_shape)
            dtype = mybir_.dt.np(alloc.dtype)
            out_avals.append(jax.core.ShapedArray(shape, dtype))
            zero_outs.append(np.zeros((N_CORES * shape[0], *shape[1:]), dtype))
    n_params = len(in_names)
    all_names = in_names + out_names
    if part_name is not None:
        all_names = all_names + [part_name]

    def _body(*args):
        operands = list(args)
        if part_name is not None:
            operands.append(partition_id_tensor())
        outs = _bass_exec_p.bind(
            *operands,
            out_avals=tuple(out_avals),
            in_names=tuple(all_names),
            out_names=tuple(out_names),
            lowering_input_output_aliases=(),
            sim_require_finite=True,
            sim_require_nnan=True,
            nc=nc,
        )
        return tuple(outs)

    devices = jax.devices()[:N_CORES]
    mesh = Mesh(np.asarray(devices), ("core",))
    spec = NamedSharding(mesh, PartitionSpec("core"))
    sharded = jax.jit(
        shard_map(
            _body,
            mesh=mesh,
            in_specs=(PartitionSpec("core"),) * (n_params + len(out_names)),
            out_specs=(PartitionSpec("core"),) * len(out_names),
            check_rep=False,
        ),
        keep_unused=True,
    )
    host_in = {"grid_before": gb, "grid_after": ga}
    dev_in = [jax.device_put(host_in[nm], spec) for nm in in_names]
    dev_zero = [jax.device_put(z, spec) for z in zero_outs]

    for _ in range(warmup):
        outs = sharded(*dev_in, *dev_zero)
    jax.block_until_ready(outs)
    t0 = time.perf_counter()
    for _ in range(iters):
        outs = sharded(*dev_in, *dev_zero)
    jax.block_until_ready(outs)
    t1 = time.perf_counter()
    per_comp_ns = (t1 - t0) / (iters * reps) * 1e9
    out = np.asarray(outs[0])
    return per_comp_ns, out


# revision 3
# speedup vs baseline: 1.3177x; 1.3177x over previous
"""Trainium2 Bass kernel: Sudoku information gain H(before) - H(after).

Self-contained: builds one SPMD Bass/Tile program, shards the batch
across 8 NeuronCores (pure data parallel), runs via
run_bass_kernel_spmd, and reassembles the full [B] output.

Algorithm per 9x9 grid (values 0..9, 0 = empty):
  encode each cell x as e = 1024 >> x on ScalarE (Exp activation):
    bit 10 <=> empty, bit (10-v) <=> value v.
  Bitwise-OR reductions of e give row/col/box presence masks on the
  DVE (u16, mostly 2x/4x-mode ops).  Per cell m = row|col|box|e_own.
  SWAR popcount: s = m - ((m>>1)&0x155) pair counts, c = base-16 digit
  compaction, pc = (c*0x1110)>>12 digit sum via u16 wrap multiply.
  fw = 8*empty comes from a parallel Relu(-16x+8) activation on
  ScalarE; u = max(fw - pc, 0) so Ln(u+1) = ln(max(9-forbidden,1)) for
  empty cells and 0 for filled ones.  Ln emits fp16; the before/after
  difference is taken per cell (2x-mode TT) so each tile needs only a
  single f32 TensorReduce; a 1/ln2 scale produces the final bits.
"""

import math
from contextlib import ExitStack

import numpy as np

import concourse.bass as bass
import concourse.bacc as bacc
import concourse.tile as tile
from concourse import mybir
from concourse.alu_op_type import AluOpType
from concourse.bass_utils import run_bass_kernel_spmd

F32 = mybir.dt.float32
F16 = mybir.dt.float16
U16 = mybir.dt.uint16
I16 = mybir.dt.int16

LN2 = math.log(2.0)
LOG1024 = math.log(1024.0)
EPS = 1e-5

OR = AluOpType.bitwise_or
AND = AluOpType.bitwise_and
ADD = AluOpType.add
SUB = AluOpType.subtract
MULT = AluOpType.mult
MAX = AluOpType.max
SHR = AluOpType.logical_shift_right

N_CORES = 8
BATCH = 262144
PER_CORE = BATCH // N_CORES  # 32768
F = 16  # grids per partition per tile

ACT_FW = True    # fw = 8*empty via Relu(-16x+8) on the Act engine
ACT_RELU = True  # u = max(t,0) via Relu on the Act engine
FP16_LN = True   # Ln output fp16, per-cell diff, single reduce per tile
MULT_PC = False  # u16 mult SATURATES on HW: use mod-15 digit sum
WP_BUFS = 3
IOP_BUFS = 3


def _masks_and_m(nc, wp, e, F):
    ve = e[:]
    e4 = ve.rearrange("p (f r c) -> p f r c", f=F, r=9, c=9)
    e5 = ve.rearrange("p (f b i c) -> p f b i c", f=F, b=3, i=3, c=9)

    t3 = wp.tile([128, F * 27], U16, tag="t3")
    t3v = t3[:].rearrange("p (f r b) -> p f r b", f=F, r=9, b=3)
    nc.vector.tensor_tensor(t3v, e4[:, :, :, 0:3], e4[:, :, :, 3:6], op=OR)
    nc.vector.tensor_tensor(t3v, t3v, e4[:, :, :, 6:9], op=OR)

    row = wp.tile([128, F * 9], U16, tag="row")
    rv = row[:].rearrange("p (f r) -> p f r", f=F, r=9)
    t3b = t3[:].rearrange("p (f r b) -> p f r b", f=F, r=9, b=3)
    nc.vector.tensor_tensor(rv, t3b[:, :, :, 0], t3b[:, :, :, 1], op=OR)
    nc.vector.tensor_tensor(rv, rv, t3b[:, :, :, 2], op=OR)

    bcol = wp.tile([128, F * 27], U16, tag="bcol")
    bv = bcol[:].rearrange("p (f b c) -> p f b c", f=F, b=3, c=9)
    nc.vector.tensor_tensor(bv, e5[:, :, :, 0, :], e5[:, :, :, 1, :], op=OR)
    nc.vector.tensor_tensor(bv, bv, e5[:, :, :, 2, :], op=OR)

    col = wp.tile([128, F * 9], U16, tag="col")
    cv = col[:].rearrange("p (f c) -> p f c", f=F, c=9)
    bc3 = bcol[:].rearrange("p (f b c) -> p f b c", f=F, b=3, c=9)
    nc.vector.tensor_tensor(cv, bc3[:, :, 0, :], bc3[:, :, 1, :], op=OR)
    nc.vector.tensor_tensor(cv, cv, bc3[:, :, 2, :], op=OR)

    box = wp.tile([128, F * 9], U16, tag="box")
    xv = box[:].rearrange("p (f b k) -> p f b k", f=F, b=3, k=3)
    bc4 = bcol[:].rearrange("p (f b k i) -> p f b k i", f=F, b=3, k=3, i=3)
    nc.vector.tensor_tensor(xv, bc4[:, :, :, :, 0], bc4[:, :, :, :, 1], op=OR)
    nc.vector.tensor_tensor(xv, xv, bc4[:, :, :, :, 2], op=OR)

    # q[f, r, bc] = row[f, r] | box[f, br(r), bc]
    q = wp.tile([128, F * 27], U16, tag="q")
    qv = q[:].rearrange("p (f b i k) -> p f b i k", f=F, b=3, i=3, k=3)
    rv3 = row[:].rearrange("p (f b i) -> p f b i", f=F, b=3, i=3)
    xv3 = box[:].rearrange("p (f b k) -> p f b k", f=F, b=3, k=3)
    for ir in range(3):
        nc.vector.tensor_tensor(
            qv[:, :, :, ir, :],
            rv3[:, :, :, ir].unsqueeze(3).broadcast_to((128, F, 3, 3)),
            xv3,
            op=OR,
        )

    # m[f, r, c] = q[f, r, bc(c)] | col[f, c]
    m = wp.tile([128, F * 81], U16, tag="m")
    mv = m[:].rearrange("p (f r b i) -> p f r b i", f=F, r=9, b=3, i=3)
    qv2 = q[:].rearrange("p (f r b) -> p f r b", f=F, r=9, b=3)
    cv2 = col[:].rearrange("p (f b i) -> p f b i", f=F, b=3, i=3)
    for bc in range(3):
        nc.vector.tensor_tensor(
            mv[:, :, :, bc, :],
            qv2[:, :, :, bc].unsqueeze(3).broadcast_to((128, F, 9, 3)),
            cv2[:, :, bc, :].unsqueeze(2).broadcast_to((128, F, 9, 3)),
            op=OR,
        )
    return m


def _entropy_from_m(nc, wp, m, e, fw_act, F):
    n = F * 81
    # pair counts: s = m - ((m>>1)&0x155); the bit-10 any-empty junk
    # never enters the masked fields (bit 0 of e is never set for 0..9)
    tmp = wp.tile([128, n], U16, tag="tmp")
    nc.vector.tensor_scalar(tmp[:], m[:], 1, 0x155, op0=SHR, op1=AND)
    s = wp.tile([128, n], U16, tag="s")
    nc.vector.tensor_tensor(s[:], m[:], tmp[:], op=SUB)

    # base-16 digit compaction: c = (s & 0x333) + ((s>>2) & 0x33)
    a = wp.tile([128, n], U16, tag="a")
    nc.vector.tensor_scalar(a[:], s[:], 2, 0x33, op0=SHR, op1=AND)
    c0 = wp.tile([128, n], U16, tag="c0")
    nc.vector.tensor_scalar(c0[:], s[:], 0x333, None, op0=AND)
    c = wp.tile([128, n], U16, tag="c")
    nc.vector.tensor_tensor(c[:], c0[:], a[:], op=ADD)

    if fw_act is not None:
        fw = fw_act
    else:
        fw = wp.tile([128, n], U16, tag="fw")
        nc.vector.tensor_scalar(fw[:], e[:], 7, 8, op0=SHR, op1=AND)

    if MULT_PC:
        # digit sum lands in bits 12-15 of c*0x1110 (u16 wrap multiply);
        # mult (arith) and shr (bitwise) cannot fuse in one tensor_scalar
        prod = wp.tile([128, n], U16, tag="prod")
        nc.vector.tensor_scalar(prod[:], c[:], 0x1110, None, op0=MULT)
        pc = wp.tile([128, n], U16, tag="pc")
        nc.vector.tensor_scalar(pc[:], prod[:], 12, None, op0=SHR)
    else:
        q15 = wp.tile([128, n], I16, tag="q15")
        nc.vector.tensor_scalar(q15[:], c[:], 1.0 / 15.0, -0.4999, op0=MULT, op1=ADD)
        v15 = wp.tile([128, n], I16, tag="v15")
        nc.vector.tensor_scalar(v15[:], q15[:], 15, None, op0=MULT)
        pc = wp.tile([128, n], I16, tag="pc")
        nc.vector.tensor_tensor(pc[:], c[:], v15[:], op=SUB)

    t = wp.tile([128, n], I16, tag="t")
    nc.vector.tensor_tensor(t[:], fw[:], pc[:], op=SUB)
    u = wp.tile([128, n], U16, tag="u")
    if ACT_RELU:
        nc.scalar.activation(u[:], t[:], mybir.ActivationFunctionType.Relu)
    else:
        nc.vector.tensor_scalar(u[:], t[:], 0, None, op0=MAX)
    return u


def _emit(tc, out_ap, gb_ap, ga_ap, n_grids, F, reps=1):
    nc = tc.nc
    per_tile = 128 * F
    n_tiles = n_grids // per_tile
    ln_dt = F16 if FP16_LN else F32

    with ExitStack() as ctx:
        cp = ctx.enter_context(tc.tile_pool(name="const", bufs=1))
        iop = ctx.enter_context(tc.tile_pool(name="io", bufs=IOP_BUFS))
        wp = ctx.enter_context(tc.tile_pool(name="work", bufs=WP_BUFS))
        accp = ctx.enter_context(tc.tile_pool(name="acc", bufs=3))

        enc_bias = cp.tile([128, 1], F32, tag="enc_bias")
        nc.vector.memset(enc_bias[:], LOG1024 + EPS)
        fw_bias = cp.tile([128, 1], F32, tag="fw_bias")
        nc.vector.memset(fw_bias[:], 8.0)

        for i in [t for _ in range(reps) for t in range(n_tiles)]:
            encoded = {}
            fws = {}
            for key, src in (("b", gb_ap), ("a", ga_ap)):
                x = iop.tile([128, F * 81], F32, tag="x")
                view = src[i * per_tile : (i + 1) * per_tile, :].rearrange(
                    "(p f) c -> p (f c)", p=128
                )
                nc.sync.dma_start(x[:], view)

                e = wp.tile([128, F * 81], U16, tag="e" + key)
                nc.scalar.activation(
                    e[:],
                    x[:],
                    mybir.ActivationFunctionType.Exp,
                    bias=enc_bias[:],
                    scale=-LN2,
                )
                encoded[key] = e
                if ACT_FW:
                    fw = wp.tile([128, F * 81], U16, tag="fw" + key)
                    nc.scalar.activation(
                        fw[:],
                        x[:],
                        mybir.ActivationFunctionType.Relu,
                        bias=fw_bias[:],
                        scale=-16.0,
                    )
                    fws[key] = fw

            us = {}
            for key in ("b", "a"):
                e = encoded[key]
                m = _masks_and_m(nc, wp, e, F)
                us[key] = _entropy_from_m(nc, wp, m, e, fws.get(key), F)

            lns = {}
            for key in ("b", "a"):
                lnv = wp.tile([128, F * 81], ln_dt, tag="ln" + key)
                nc.scalar.activation(
                    lnv[:], us[key][:], mybir.ActivationFunctionType.Ln, bias=1.0
                )
                lns[key] = lnv

            if FP16_LN:
                lnd = wp.tile([128, F * 81], F16, tag="lnd")
                nc.vector.tensor_tensor(lnd[:], lns["b"][:], lns["a"][:], op=SUB)
                tot = accp.tile([128, F], F32, tag="tot")
                nc.vector.tensor_reduce(
                    tot[:],
                    lnd[:].rearrange("p (f c) -> p f c", f=F, c=81),
                    axis=mybir.AxisListType.X,
                    op=ADD,
                )
                nc.vector.tensor_scalar(tot[:], tot[:], 1.0 / LN2, None, op0=MULT)
                diff = tot
            else:
                tots = {}
                for key in ("b", "a"):
                    tot = accp.tile([128, F], F32, tag="tot" + key)
                    nc.vector.tensor_reduce(
                        tot[:],
                        lns[key][:].rearrange("p (f c) -> p f c", f=F, c=81),
                        axis=mybir.AxisListType.X,
                        op=ADD,
                    )
                    tots[key] = tot
                diff = accp.tile([128, F], F32, tag="diff")
                nc.vector.tensor_tensor(diff[:], tots["b"][:], tots["a"][:], op=SUB)
                nc.vector.tensor_scalar(diff[:], diff[:], 1.0 / LN2, None, op0=MULT)

            out_view = out_ap[i * per_tile : (i + 1) * per_tile].rearrange(
                "(p f) -> p f", p=128
            )
            nc.sync.dma_start(out_view, diff[:])


_PROGRAM_CACHE = {}


def _build_program(reps=1):
    key = (PER_CORE, F, reps, ACT_FW, ACT_RELU, FP16_LN, MULT_PC, WP_BUFS, IOP_BUFS)
    if key in _PROGRAM_CACHE:
        return _PROGRAM_CACHE[key]
    nc = bacc.Bacc("TRN2", target_bir_lowering=False, debug=False)
    gb = nc.dram_tensor("grid_before", [PER_CORE, 81], F32, kind="ExternalInput")
    ga = nc.dram_tensor("grid_after", [PER_CORE, 81], F32, kind="ExternalInput")
    out = nc.dram_tensor("out", [PER_CORE], F32, kind="ExternalOutput")
    with tile.TileContext(nc) as tc:
        _emit(tc, out.ap(), gb.ap(), ga.ap(), PER_CORE, F, reps=reps)
    nc.finalize()
    _PROGRAM_CACHE[key] = nc
    return nc


def run(grid_before, grid_after, trace=False, **trace_kwargs):
    gb = np.ascontiguousarray(
        np.asarray(grid_before, dtype=np.float32).reshape(BATCH, 81)
    )
    ga = np.ascontiguousarray(
        np.asarray(grid_after, dtype=np.float32).reshape(BATCH, 81)
    )
    nc = _build_program()
    in_maps = [
        {
            "grid_before": gb[k * PER_CORE : (k + 1) * PER_CORE],
            "grid_after": ga[k * PER_CORE : (k + 1) * PER_CORE],
        }
        for k in range(N_CORES)
    ]
    res = run_bass_kernel_spmd(
        nc, in_maps, list(range(N_CORES)), trace=trace, **trace_kwargs
    )
    out = np.concatenate([res.results[k]["out"] for k in range(N_CORES)])
    return out, res


def kernel(grid_before, grid_after):
    out, _ = run(grid_before, grid_after)
    return out


def bench(grid_before, grid_after, iters=250, warmup=3, reps=8):
    """Per-computation wall time with device-resident inputs.

    The NEFF repeats the full computation `reps` times back-to-back so
    the per-dispatch axon RPC cost (~1 ms) is amortized below the
    kernel's own execution time; `iters` pipelined dispatches amortize
    the fixed pipeline-fill cost.  Reported time = wall / (iters*reps).
    """
    import time

    import jax
    import concourse.mybir as mybir_
    from jax.sharding import Mesh, NamedSharding, PartitionSpec
    from jax.experimental.shard_map import shard_map
    from concourse.bass2jax import (
        _bass_exec_p,
        install_neuronx_cc_hook,
        partition_id_tensor,
    )

    install_neuronx_cc_hook()
    gb = np.ascontiguousarray(
        np.asarray(grid_before, dtype=np.float32).reshape(BATCH, 81)
    )
    ga = np.ascontiguousarray(
        np.asarray(grid_after, dtype=np.float32).reshape(BATCH, 81)
    )
    nc = _build_program(reps=reps)

    part_name = nc.partition_id_tensor.name if nc.partition_id_tensor else None
    in_names, out_names, out_avals, zero_outs = [], [], [], []
    for alloc in nc.m.functions[0].allocations:
        if not isinstance(alloc, mybir.MemoryLocationSet):
            continue
        name = alloc.memorylocations[0].name
        if alloc.kind == "ExternalInput":
            if name != part_name:
                in_names.append(name)
        elif alloc.kind == "ExternalOutput":
            out_names.append(name)
            shape = tuple(alloc.tensor_shape)
            dtype = mybir_.dt.np(alloc.dtype)
            out_avals.append(jax.core.ShapedArray(shape, dtype))
            zero_outs.append(np.zeros((N_CORES * shape[0], *shape[1:]), dtype))
    n_params = len(in_names)
    all_names = in_names + out_names
    if part_name is not None:
        all_names = all_names + [part_name]

    def _body(*args):
        operands = list(args)
        if part_name is not None:
            operands.append(partition_id_tensor())
        outs = _bass_exec_p.bind(
            *operands,
            out_avals=tuple(out_avals),
            in_names=tuple(all_names),
            out_names=tuple(out_names),
            lowering_input_output_aliases=(),
            sim_require_finite=True,
            sim_require_nnan=True,
            nc=nc,
        )
        return tuple(outs)

    devices = jax.devices()[:N_CORES]
    mesh = Mesh(np.asarray(devices), ("core",))
    spec = NamedSharding(mesh, PartitionSpec("core"))
    sharded = jax.jit(
        shard_map(
            _body,
            mesh=mesh,
            in_specs=(PartitionSpec("core"),) * (n_params + len(out_names)),
            out_specs=(PartitionSpec("core"),) * len(out_names),
            check_rep=False,
        ),
        keep_unused=True,
    )
    host_in = {"grid_before": gb, "grid_after": ga}
    dev_in = [jax.device_put(host_in[nm], spec) for nm in in_names]
    dev_zero = [jax.device_put(z, spec) for z in zero_outs]

    for _ in range(warmup):
        outs = sharded(*dev_in, *dev_zero)
    jax.block_until_ready(outs)
    t0 = time.perf_counter()
    for _ in range(iters):
        outs = sharded(*dev_in, *dev_zero)
    jax.block_until_ready(outs)
    t1 = time.perf_counter()
    per_comp_ns = (t1 - t0) / (iters * reps) * 1e9
    out = np.asarray(outs[0])
    return per_comp_ns, out
